# revision 18
# baseline (speedup 1.0000x reference)
"""HANLayer (2x GATConv + semantic attention) Trainium2 Bass kernel, 8 cores.

Strategy (v3): edges sorted by dst, dst-node-sharded aggregation (1280 dst
nodes/core), replicated dense projection split per conv so conv2's projection
overlaps conv1's gather/aggregate.  Attention source logits are embedded in
the gathered H rows (width 1152 = 1024 h | 1 s | pad); destination logits are
expanded per chunk with a tiny matmul against a host-provided transposed
one-hot (EMTT), so each gather group needs exactly ONE dma_gather (1024 rows).
Per-128-edge one-hot matmuls do the softmax-weighted segment sum.
"""
import os
import sys

for _p in ("/opt/trn_rl_repo", "/root/.axon_site/_ro/trn_rl_repo"):
    if os.path.isdir(_p) and _p not in sys.path:
        sys.path.insert(0, _p)

import numpy as np
import ml_dtypes

import concourse.bacc as bacc
import concourse.bass as bass
import concourse.mybir as mybir
import concourse.tile as tile
from concourse import bass_utils, library_config
from concourse.masks import make_identity

F32 = mybir.dt.float32
BF16 = mybir.dt.bfloat16
I16 = mybir.dt.int16

N = 10000
E = 160000
IN_C = 512
OUT_C = 1024
NEG_SLOPE = 0.2
NCORES = 8
NPAD = 10240            # N padded to 80 blocks of 128
BPC = 10                # dst blocks per core
NODES_PER_CORE = 1280
P = 128
G = 8                   # chunks per gather group (1024 idx = HW limit)
NB = 80                 # node blocks
HW_W = 1152             # H row: 1024 h | 1 s | 127 pad (2304 B, /256 ok)

AddOp = mybir.AluOpType.add
SubOp = mybir.AluOpType.subtract
MulOp = mybir.AluOpType.mult
MaxOp = mybir.AluOpType.max


def _wrap_idx16(flat):
    """int16 index layout for dma_gather: idx i at [16*rep + i%16, i//16]."""
    n = flat.shape[0]
    assert n % 16 == 0
    w = flat.astype(np.int16).reshape(n // 16, 16).T  # [16, n//16]
    return np.tile(w, (8, 1))  # [128, n//16]


def _host_prep(edge_index):
    """Sort edges (plus self loops incl. pad nodes) by dst, chunk per core."""
    src = np.concatenate([edge_index[0].astype(np.int64),
                          np.arange(NPAD, dtype=np.int64)])
    dst = np.concatenate([edge_index[1].astype(np.int64),
                          np.arange(NPAD, dtype=np.int64)])
    order = np.argsort(dst, kind="stable")
    src_s = src[order]
    dst_s = dst[order]
    blk = dst_s // P
    counts = np.bincount(blk, minlength=NB)
    K = int(np.ceil(counts.max() / P))
    C = BPC * K
    C_pad = ((C + G - 1) // G) * G
    src_idx = np.zeros((NCORES, C_pad, P), np.int64)
    em = np.zeros((NCORES, C_pad, P, P), np.float32)
    bstart = np.searchsorted(blk, np.arange(NB + 1))
    for b in range(NB):
        core, bslot = divmod(b, BPC)
        lo, hi = bstart[b], bstart[b + 1]
        nb_ = hi - lo
        es = src_s[lo:hi]
        dloc = (dst_s[lo:hi] - b * P).astype(np.int64)
        for c in range((nb_ + P - 1) // P):
            e0 = c * P
            e1 = min(e0 + P, nb_)
            ci = bslot * K + c
            npts = e1 - e0
            src_idx[core, ci, :npts] = es[e0:e1]
            em[core, ci, np.arange(npts), dloc[e0:e1]] = 1.0
    NG = C_pad // G
    # group-major layouts:
    # EMTG[core, g, e, c*128+d] = em[core, g*G+c, e, d]
    emtg = np.ascontiguousarray(
        em.reshape(NCORES, NG, G, P, P).transpose(0, 1, 3, 2, 4)
        .reshape(NCORES, NG, P, G * P))
    # EMTT[core, g, d, c*128+e] = em[core, g*G+c, e, d]
    emtt = np.ascontiguousarray(
        em.transpose(0, 1, 3, 2).reshape(NCORES, NG, G, P, P)
        .transpose(0, 1, 3, 2, 4).reshape(NCORES, NG, P, G * P))
    sidx16 = np.stack([_wrap_idx16(src_idx[c].reshape(-1))
                       for c in range(NCORES)])
    dn16 = np.stack([_wrap_idx16(np.arange(c * NODES_PER_CORE,
                                           (c + 1) * NODES_PER_CORE))
                     for c in range(NCORES)])
    return K, sidx16, emtg, emtt, dn16


def _build_program(K, debug=False):
    C = BPC * K
    C_pad = ((C + G - 1) // G) * G
    NG = C_pad // G
    KIND_DBG = "ExternalOutput" if debug else "Internal"
    nc = bacc.Bacc("TRN2", target_bir_lowering=False, debug=False,
                   enable_asserts=False, num_devices=NCORES)

    # ---- inputs (replicated except EMTG/EMTT/SIDX16/MSK) ----
    XT = nc.dram_tensor("XT", [IN_C, NPAD], BF16, kind="ExternalInput")
    W1C = nc.dram_tensor("W1C", [IN_C, OUT_C], BF16, kind="ExternalInput")
    W2C = nc.dram_tensor("W2C", [IN_C, OUT_C], BF16, kind="ExternalInput")
    W1T = nc.dram_tensor("W1T", [OUT_C, IN_C], F32, kind="ExternalInput")
    W2T = nc.dram_tensor("W2T", [OUT_C, IN_C], F32, kind="ExternalInput")
    A4 = nc.dram_tensor("A4", [OUT_C, 4], F32, kind="ExternalInput")
    B1 = nc.dram_tensor("B1", [1, OUT_C], F32, kind="ExternalInput")
    B2 = nc.dram_tensor("B2", [1, OUT_C], F32, kind="ExternalInput")
    BP1C = nc.dram_tensor("BP1C", [P, 8], F32, kind="ExternalInput")
    PRA = nc.dram_tensor("PRA", [1, 1], F32, kind="ExternalInput")
    MSK = nc.dram_tensor("MSK", [1, 1], F32, kind="ExternalInput")
    WP1 = nc.dram_tensor("WP1", [OUT_C, OUT_C], BF16, kind="ExternalInput")
    WP2 = nc.dram_tensor("WP2", [OUT_C, OUT_C], BF16, kind="ExternalInput")
    EMTG = nc.dram_tensor("EMTG", [NG, P, G * P], BF16, kind="ExternalInput")
    EMTT = nc.dram_tensor("EMTT", [NG, P, G * P], BF16, kind="ExternalInput")
    SIDX16 = nc.dram_tensor("SIDX16", [P, C_pad * 8], I16,
                            kind="ExternalInput")
    DN16 = nc.dram_tensor("DN16", [P, BPC * 8], I16, kind="ExternalInput")

    OUT = nc.dram_tensor("OUT", [NODES_PER_CORE, OUT_C], F32,
                         kind="ExternalOutput")

    # ---- internal DRAM ----
    H1 = nc.dram_tensor("H1", [NPAD, HW_W], BF16, kind=KIND_DBG)
    H2 = nc.dram_tensor("H2", [NPAD, HW_W], BF16, kind=KIND_DBG)
    S4 = nc.dram_tensor("S4", [NPAD, 64], F32, kind=KIND_DBG)
    ARIN = nc.dram_tensor("ARIN", [OUT_C], F32, kind="Internal")
    AROUT = nc.dram_tensor("AROUT", [OUT_C], F32, kind="Internal",
                           addr_space="Shared")
    ATTD = nc.dram_tensor("ATTD", [1, OUT_C], F32, kind=KIND_DBG)
    if debug:
        DBGH1 = nc.dram_tensor("DBGH1", [NODES_PER_CORE, OUT_C], F32,
                               kind="ExternalOutput")
        DBGH2 = nc.dram_tensor("DBGH2", [NODES_PER_CORE, OUT_C], F32,
                               kind="ExternalOutput")

    NKC = IN_C // P  # 4 k-chunks of input features

    with tile.TileContext(nc) as tc:
        with tc.tile_pool(name="persist", bufs=1) as pp:
            nc.gpsimd.load_library(library_config.mlp)
            b1b = pp.tile([P, OUT_C], F32, tag="b1b")
            b2b = pp.tile([P, OUT_C], F32, tag="b2b")
            nc.sync.dma_start(b1b[:], B1.ap().to_broadcast((P, OUT_C)))
            nc.sync.dma_start(b2b[:], B2.ap().to_broadcast((P, OUT_C)))
            bp1c = pp.tile([P, 8], F32, tag="bp1c")
            nc.sync.dma_start(bp1c[:], BP1C.ap())
            pa_col = pp.tile([P, 1], F32, tag="pa_col")
            nc.sync.dma_start(pa_col[:], PRA.ap().to_broadcast((P, 1)))
            msk_col = pp.tile([P, 1], F32, tag="msk_col")
            nc.sync.dma_start(msk_col[:], MSK.ap().to_broadcast((P, 1)))
            ones = pp.tile([P, 1], BF16, tag="ones")
            nc.vector.memset(ones[:], 1.0)
            identity = pp.tile([P, P], F32, tag="identity")
            make_identity(nc, identity[:])
            identity16 = pp.tile([P, P], BF16, tag="identity16")
            nc.vector.tensor_copy(identity16[:], identity[:])
            sidx16 = pp.tile([P, C_pad * 8], I16, tag="sidx16")
            nc.sync.dma_start(sidx16[:], SIDX16.ap())
            dn16 = pp.tile([P, BPC * 8], I16, tag="dn16")
            nc.sync.dma_start(dn16[:], DN16.ap())
            acc1 = pp.tile([P, BPC * OUT_C], F32, tag="acc1")
            acc2 = pp.tile([P, BPC * OUT_C], F32, tag="acc2")
            tbcols = pp.tile([P, 96], F32, tag="tbcols")
            wt_g = [pp.tile([P, 4], BF16, tag=f"wt{g}", name=f"wt{g}")
                    for g in range(NKC)]

            # ================= wtilde = [W1a, W2a] =================
            with tc.tile_pool(name="wt", bufs=1) as sp, \
                 tc.tile_pool(name="wtps", bufs=1, space="PSUM") as psp:
                a4t = [sp.tile([P, 4], F32, tag="a4", bufs=8, name=f"a4_{oc}")
                       for oc in range(8)]
                for oc in range(8):
                    nc.sync.dma_start(a4t[oc][:],
                                      A4.ap()[oc * P:(oc + 1) * P, :])
                for g in range(NKC):
                    pwt1 = psp.tile([P, 2], F32, tag="pwt", bufs=4)
                    pwt2 = psp.tile([P, 2], F32, tag="pwt", bufs=4)
                    for oc in range(8):
                        w1t_t = sp.tile([P, P], F32, tag="w1t", bufs=3)
                        nc.sync.dma_start(
                            w1t_t[:], W1T.ap()[oc * P:(oc + 1) * P,
                                               g * P:(g + 1) * P])
                        nc.tensor.matmul(pwt1[:], lhsT=w1t_t[:],
                                         rhs=a4t[oc][:, 0:2],
                                         start=(oc == 0), stop=(oc == 7))
                        w2t_t = sp.tile([P, P], F32, tag="w2t", bufs=3)
                        nc.sync.dma_start(
                            w2t_t[:], W2T.ap()[oc * P:(oc + 1) * P,
                                               g * P:(g + 1) * P])
                        nc.tensor.matmul(pwt2[:], lhsT=w2t_t[:],
                                         rhs=a4t[oc][:, 2:4],
                                         start=(oc == 0), stop=(oc == 7))
                    nc.vector.tensor_copy(wt_g[g][:, 0:2], pwt1[:])
                    nc.vector.tensor_copy(wt_g[g][:, 2:4], pwt2[:])

            # ========== P1a: conv1 projection + S ==========
            with tc.tile_pool(name="p1a", bufs=1) as sp, \
                 tc.tile_pool(name="p1aps", bufs=1, space="PSUM") as psp:
                rhs1 = [sp.tile([P, OUT_C], BF16, tag=f"rhs1_{g}",
                                name=f"rhs1_{g}") for g in range(NKC)]
                for g in range(NKC):
                    nc.sync.dma_start(rhs1[g][:],
                                      W1C.ap()[g * P:(g + 1) * P, :])
                xtc = None
                for i in range(NB):
                    i4, j = divmod(i, 4)
                    if j == 0:
                        xtc = sp.tile([P, NKC, 4 * P], BF16, tag="xtc",
                                      bufs=2)
                        nc.sync.dma_start(
                            xtc[:, :, :],
                            bass.AP(XT, i4 * 4 * P,
                                    [[NPAD, P], [P * NPAD, NKC], [1, 4 * P]]))
                    ps_h = psp.tile([P, OUT_C], F32, tag="ph", bufs=2)
                    ps_s = psp.tile([P, 4], F32, tag="ps", bufs=2)
                    for g in range(NKC):
                        lh = xtc[:, g, j * P:(j + 1) * P]
                        nc.tensor.matmul(ps_h[:, 0:512], lhsT=lh,
                                         rhs=rhs1[g][:, 0:512],
                                         start=(g == 0), stop=(g == NKC - 1))
                        nc.tensor.matmul(ps_h[:, 512:1024], lhsT=lh,
                                         rhs=rhs1[g][:, 512:1024],
                                         start=(g == 0), stop=(g == NKC - 1))
                        nc.tensor.matmul(ps_s[:], lhsT=lh,
                                         rhs=wt_g[g][:, 0:4],
                                         start=(g == 0), stop=(g == NKC - 1))
                    stg = sp.tile([P, HW_W], BF16, tag="stg", bufs=3)
                    nc.vector.memset(stg[:, OUT_C + 1:HW_W], 0.0)
                    nc.vector.tensor_copy(stg[:, 0:OUT_C], ps_h[:])
                    nc.vector.tensor_copy(stg[:, OUT_C:OUT_C + 1],
                                          ps_s[:, 0:1])
                    s64 = sp.tile([P, 64], F32, tag="s64", bufs=3)
                    nc.vector.memset(s64[:, 4:64], 0.0)
                    nc.vector.tensor_copy(s64[:, 0:4], ps_s[:])
                    rows = slice(i * P, (i + 1) * P)
                    nc.sync.dma_start(H1.ap()[rows, :], stg[:])
                    nc.sync.dma_start(S4.ap()[rows, :], s64[:])

            # ========== agg phase (shared for both convs) ==========
            def agg_phase(sp, psp, Hsrc, soff, accT, bbT, proj_emitter,
                          proj_total, post_block):
                proj_done = [0]
                num = [None]
                den = [None]
                sbd = sp.tile([P, BPC, 64], F32, tag="sbd", name="sbd")
                nc.gpsimd.dma_gather(
                    sbd[:, 0:5, :], S4.ap(), dn16[:, 0:40],
                    5 * P, 5 * P, 64)
                nc.gpsimd.dma_gather(
                    sbd[:, 5:10, :], S4.ap(), dn16[:, 40:80],
                    5 * P, 5 * P, 64)
                sb16 = sp.tile([P, BPC], BF16, tag="sb16", name="sb16")
                nc.vector.tensor_copy(sb16[:], sbd[:, :, soff:soff + 1])
                for gi in range(NG):
                    target = min(proj_total,
                                 int(np.ceil(proj_total * (gi + 1) / NG)))
                    while proj_done[0] < target:
                        proj_emitter(proj_done[0])
                        proj_done[0] += 1
                    hg = sp.tile([P, G, HW_W], BF16, tag="hg", bufs=2)
                    nc.gpsimd.dma_gather(
                        hg[:, :, :], Hsrc.ap(),
                        sidx16[:, gi * G * 8:(gi + 1) * G * 8],
                        G * P, G * P, HW_W)
                    emg = sp.tile([P, G * P], BF16, tag="emg", bufs=2)
                    nc.scalar.dma_start(emg[:], EMTG.ap()[gi])
                    emt = sp.tile([P, G * P], BF16, tag="emt", bufs=2)
                    nc.scalar.dma_start(emt[:], EMTT.ap()[gi])
                    # dst-side logits for each chunk via transposed one-hot
                    dvps = psp.tile([P, G], F32, tag="dvps", bufs=1)
                    nc.vector.memset(dvps[:], 0.0)
                    for c in range(G):
                        ci = gi * G + c
                        if ci >= C:
                            continue
                        b_ = ci // K
                        nc.tensor.matmul(dvps[:, c:c + 1],
                                         lhsT=emt[:, c * P:(c + 1) * P],
                                         rhs=sb16[:, b_:b_ + 1],
                                         start=True, stop=True)
                    e2 = sp.tile([P, G], F32, tag="e2", bufs=2)
                    nc.vector.tensor_tensor(
                        out=e2[:], in0=hg[:, :, OUT_C:OUT_C + 1],
                        in1=dvps[:], op=AddOp)
                    lr = sp.tile([P, G], F32, tag="lr", bufs=2)
                    nc.vector.scalar_tensor_tensor(
                        out=lr[:], in0=e2[:], scalar=NEG_SLOPE, in1=e2[:],
                        op0=MulOp, op1=MaxOp)
                    al = sp.tile([P, G], F32, tag="al", bufs=2)
                    nc.scalar.activation(al[:], lr[:],
                                         mybir.ActivationFunctionType.Exp)
                    for c in range(G):
                        ci = gi * G + c
                        if ci >= C:
                            continue
                        b, k = divmod(ci, K)
                        am = sp.tile([P, P], BF16, tag="am", bufs=3)
                        nc.vector.tensor_scalar_mul(
                            am[:], emg[:, c * P:(c + 1) * P], al[:, c:c + 1])
                        if k == 0:
                            num[0] = psp.tile([P, OUT_C], F32, tag="num",
                                              bufs=1, name="num")
                            den[0] = psp.tile([P, 1], F32, tag="den", bufs=1,
                                              name="den")
                        nc.tensor.matmul(num[0][:, 0:512], lhsT=am[:],
                                         rhs=hg[:, c, 0:512],
                                         start=(k == 0), stop=(k == K - 1))
                        nc.tensor.matmul(num[0][:, 512:1024], lhsT=am[:],
                                         rhs=hg[:, c, 512:1024],
                                         start=(k == 0), stop=(k == K - 1))
                        nc.tensor.matmul(den[0][:], lhsT=am[:], rhs=ones[:],
                                         start=(k == 0), stop=(k == K - 1))
                        if k == K - 1:
                            rden = sp.tile([P, 1], F32, tag="rden", bufs=2)
                            nc.vector.reciprocal(rden[:], den[0][:])
                            tmp = sp.tile([P, OUT_C], F32, tag="gtmp", bufs=2)
                            nc.vector.scalar_tensor_tensor(
                                out=tmp[:], in0=num[0][:],
                                scalar=rden[:, 0:1], in1=bbT[:],
                                op0=MulOp, op1=AddOp)
                            hcols = slice(b * OUT_C, (b + 1) * OUT_C)
                            nc.vector.scalar_tensor_tensor(
                                out=accT[:, hcols], in0=tmp[:],
                                scalar=pa_col[:, 0:1], in1=tmp[:],
                                op0=MulOp, op1=MaxOp)
                            post_block(b)
                while proj_done[0] < proj_total:
                    proj_emitter(proj_done[0])
                    proj_done[0] += 1

            # ========== P2a (conv1 agg) overlapped with P1b (conv2 proj) ====
            with tc.tile_pool(name="p2a", bufs=1) as sp, \
                 tc.tile_pool(name="p2aps", bufs=1, space="PSUM") as psp:
                rhs2 = [sp.tile([P, OUT_C], BF16, tag=f"rhs2_{g}",
                                name=f"rhs2_{g}") for g in range(NKC)]
                for g in range(NKC):
                    nc.sync.dma_start(rhs2[g][:],
                                      W2C.ap()[g * P:(g + 1) * P, :])
                xtc2 = [None]

                def proj2(i):
                    i4, j = divmod(i, 4)
                    if j == 0:
                        xtc2[0] = sp.tile([P, NKC, 4 * P], BF16, tag="xtc2",
                                          bufs=2, name="xtc2")
                        nc.sync.dma_start(
                            xtc2[0][:, :, :],
                            bass.AP(XT, i4 * 4 * P,
                                    [[NPAD, P], [P * NPAD, NKC], [1, 4 * P]]))
                    ps_h = psp.tile([P, OUT_C], F32, tag="ph2", bufs=2)
                    for g in range(NKC):
                        lh = xtc2[0][:, g, j * P:(j + 1) * P]
                        nc.tensor.matmul(ps_h[:, 0:512], lhsT=lh,
                                         rhs=rhs2[g][:, 0:512],
                                         start=(g == 0), stop=(g == NKC - 1))
                        nc.tensor.matmul(ps_h[:, 512:1024], lhsT=lh,
                                         rhs=rhs2[g][:, 512:1024],
                                         start=(g == 0), stop=(g == NKC - 1))
                    stg = sp.tile([P, HW_W], BF16, tag="stg2", bufs=3,
                                  name="stg2")
                    s2col = sp.tile([P, 1], F32, tag="s2col", bufs=2,
                                    name="s2col")
                    nc.sync.dma_start(s2col[:],
                                      S4.ap()[i * P:(i + 1) * P, 2:3])
                    nc.vector.memset(stg[:, OUT_C + 1:HW_W], 0.0)
                    nc.vector.tensor_copy(stg[:, 0:OUT_C], ps_h[:])
                    nc.vector.tensor_copy(stg[:, OUT_C:OUT_C + 1], s2col[:])
                    nc.sync.dma_start(H2.ap()[i * P:(i + 1) * P, :], stg[:])

                agg_phase(sp, psp, H1, 1, acc1, b1b, proj2, NB,
                          lambda b: None)

            # ========== P2b (conv2 agg) overlapped with P3 per-block ========
            with tc.tile_pool(name="p2b", bufs=1) as sp, \
                 tc.tile_pool(name="p2bps", bufs=1, space="PSUM") as psp:
                wp1k = [sp.tile([P, OUT_C], BF16, tag=f"wp1_{k}",
                                name=f"wp1_{k}") for k in range(8)]
                for k in range(8):
                    nc.scalar.dma_start(wp1k[k][:],
                                        WP1.ap()[k * P:(k + 1) * P, :])

                def p3_block(b):
                    hcols = slice(b * OUT_C, (b + 1) * OUT_C)
                    hsb = sp.tile([P, OUT_C], BF16, tag="hsb", bufs=2)
                    nc.vector.tensor_tensor(out=hsb[:], in0=acc1[:, hcols],
                                            in1=acc2[:, hcols], op=AddOp)
                    htb = []
                    for kk in range(8):
                        tp = psp.tile([P, P], BF16, tag="tp", bufs=2)
                        nc.tensor.transpose(tp[:],
                                            hsb[:, kk * P:(kk + 1) * P],
                                            identity16[:])
                        ht = sp.tile([P, P], BF16, tag=f"htb{kk}", bufs=2,
                                     name=f"htb{kk}")
                        nc.vector.tensor_copy(ht[:], tp[:])
                        htb.append(ht)
                    for o2 in range(8):
                        wps = psp.tile([P, P], F32, tag="wps", bufs=2)
                        for kk in range(8):
                            nc.tensor.matmul(
                                wps[:],
                                lhsT=wp1k[kk][:, o2 * P:(o2 + 1) * P],
                                rhs=htb[kk][:],
                                start=(kk == 0), stop=(kk == 7))
                        scr = sp.tile([P, P], BF16, tag="scr", bufs=2)
                        col0 = o2 * 12
                        if b < 8:
                            nc.scalar.activation(
                                scr[:], wps[:],
                                mybir.ActivationFunctionType.Tanh,
                                bias=bp1c[:, o2:o2 + 1],
                                accum_out=tbcols[:, col0 + b:col0 + b + 1])
                        elif b == 8:
                            nc.scalar.activation(
                                scr[:, 0:16], wps[:, 0:16],
                                mybir.ActivationFunctionType.Tanh,
                                bias=bp1c[:, o2:o2 + 1],
                                accum_out=tbcols[:, col0 + 8:col0 + 9])
                            nc.scalar.activation(
                                scr[:, 16:P], wps[:, 16:P],
                                mybir.ActivationFunctionType.Tanh,
                                bias=bp1c[:, o2:o2 + 1],
                                accum_out=tbcols[:, col0 + 9:col0 + 10])
                        else:
                            nc.scalar.activation(
                                scr[:], wps[:],
                                mybir.ActivationFunctionType.Tanh,
                                bias=bp1c[:, o2:o2 + 1],
                                accum_out=tbcols[:, col0 + 10:col0 + 11])

                agg_phase(sp, psp, H2, 3, acc2, b2b, lambda i: None, 0,
                          p3_block)

            # ========== tail: semantic attention + blend ==========
            with tc.tile_pool(name="tail", bufs=1) as sp, \
                 tc.tile_pool(name="tailps", bufs=1, space="PSUM") as psp:
                wp2k = [sp.tile([P, OUT_C], BF16, tag=f"wp2_{k}",
                                name=f"wp2_{k}") for k in range(8)]
                for k in range(8):
                    nc.scalar.dma_start(wp2k[k][:],
                                        WP2.ap()[k * P:(k + 1) * P, :])
                tbar = sp.tile([P, 8], F32, tag="tbar")
                for o2 in range(8):
                    col0 = o2 * 12
                    tmain = sp.tile([P, 1], F32, tag="tmain", bufs=2)
                    nc.vector.tensor_reduce(
                        tmain[:], tbcols[:, col0:col0 + 9],
                        mybir.AxisListType.X, AddOp)
                    tmsk = sp.tile([P, 1], F32, tag="tmsk", bufs=2)
                    nc.vector.tensor_reduce(
                        tmsk[:], tbcols[:, col0 + 9:col0 + 11],
                        mybir.AxisListType.X, AddOp)
                    nc.vector.scalar_tensor_tensor(
                        out=tbar[:, o2:o2 + 1], in0=tmsk[:],
                        scalar=msk_col[:, 0:1], in1=tmain[:],
                        op0=MulOp, op1=AddOp)
                arview = [[1, P], [P, 8]]
                nc.sync.dma_start(bass.AP(ARIN, 0, arview), tbar[:])
                nc.gpsimd.collective_compute(
                    "AllReduce", AddOp,
                    replica_groups=[list(range(NCORES))],
                    ins=[ARIN.ap().opt()], outs=[AROUT.ap().opt()])
                tbm = sp.tile([P, 8], F32, tag="tbm")
                nc.sync.dma_start(tbm[:], bass.AP(AROUT, 0, arview))
                tbn = sp.tile([P, 8], BF16, tag="tbn")
                nc.vector.tensor_scalar_mul(tbn[:], tbm[:], 1.0 / N)
                pw = psp.tile([1, OUT_C], F32, tag="pw")
                for kk in range(8):
                    nc.tensor.matmul(pw[:, 0:512], lhsT=tbn[:, kk:kk + 1],
                                     rhs=wp2k[kk][:, 0:512], start=(kk == 0),
                                     stop=(kk == 7))
                    nc.tensor.matmul(pw[:, 512:1024], lhsT=tbn[:, kk:kk + 1],
                                     rhs=wp2k[kk][:, 512:1024],
                                     start=(kk == 0), stop=(kk == 7))
                et = sp.tile([1, OUT_C], F32, tag="et")
                esum = sp.tile([1, 1], F32, tag="esum")
                nc.scalar.activation(et[:], pw[:],
                                     mybir.ActivationFunctionType.Exp,
                                     accum_out=esum[:])
                rs = sp.tile([1, 1], F32, tag="rs")
                nc.vector.reciprocal(rs[:], esum[:])
                att1 = sp.tile([1, OUT_C], F32, tag="att1")
                nc.vector.tensor_scalar_mul(att1[:], et[:], rs[:, 0:1])
                nc.sync.dma_start(ATTD.ap(), att1[:])
                attb = sp.tile([P, OUT_C], F32, tag="attb")
                nc.sync.dma_start(attb[:],
                                  ATTD.ap().to_broadcast((P, OUT_C)))
                for b in range(BPC):
                    hcols = slice(b * OUT_C, (b + 1) * OUT_C)
                    d = sp.tile([P, OUT_C], F32, tag="bd", bufs=2)
                    nc.vector.tensor_tensor(out=d[:], in0=acc1[:, hcols],
                                            in1=acc2[:, hcols], op=SubOp)
                    m = sp.tile([P, OUT_C], F32, tag="bm", bufs=2)
                    nc.vector.tensor_tensor(out=m[:], in0=d[:], in1=attb[:],
                                            op=MulOp)
                    o = sp.tile([P, OUT_C], F32, tag="bo", bufs=2)
                    nc.vector.tensor_tensor(out=o[:], in0=m[:],
                                            in1=acc2[:, hcols], op=AddOp)
                    nc.sync.dma_start(OUT.ap()[b * P:(b + 1) * P, :], o[:])
                if debug:
                    for b in range(BPC):
                        hcols = slice(b * OUT_C, (b + 1) * OUT_C)
                        nc.sync.dma_start(
                            DBGH1.ap()[b * P:(b + 1) * P, :], acc1[:, hcols])
                        nc.sync.dma_start(
                            DBGH2.ap()[b * P:(b + 1) * P, :], acc2[:, hcols])

    nc.compile()
    return nc


_PROG_CACHE = {}


def _ensure_trace_support():
    """Install the missing antenv.axon_hooks NTFF shim so trace=True works."""
    import types
    try:
        from antenv import axon_hooks  # noqa: F401
        return True
    except ImportError:
        pass
    try:
        import antenv
        if "/root/.axon_site" not in sys.path:
            sys.path.append("/root/.axon_site")
        from trn_agent_boot.trn_boot import _ntff_profile_via_ctypes
        hook = _ntff_profile_via_ctypes("/opt/axon/libaxon_pjrt.so")
        if hook is None:
            return False
        mod = types.ModuleType("antenv.axon_hooks")
        mod._hook = hook
        mod.get_axon_ntff_profile_hook = lambda: mod._hook
        mod.set_axon_ntff_profile_hook = lambda h: setattr(mod, "_hook", h)
        sys.modules["antenv.axon_hooks"] = mod
        antenv.axon_hooks = mod
        bass_utils.upload_artifacts = lambda t: str(t)
        return True
    except Exception as e:  # noqa: BLE001
        print("trace support unavailable:", e)
        return False


def _get_program(K):
    if K not in _PROG_CACHE:
        _PROG_CACHE[K] = _build_program(K)
    return _PROG_CACHE[K]


def _run(inputs, trace=False, debug=False, tmpdir=None):
    x = np.asarray(inputs["x"], np.float32)
    edge_index = np.asarray(inputs["edge_index"])
    K, sidx16, emtg, emtt, dn16 = _host_prep(edge_index)
    if debug:
        nc = _build_program(K, debug=True)
    else:
        nc = _get_program(K)

    xpad = np.zeros((NPAD, IN_C), np.float32)
    xpad[:N] = x
    xT = np.ascontiguousarray(xpad.T).astype(ml_dtypes.bfloat16)
    W1f = np.ascontiguousarray(np.asarray(inputs["W1"], np.float32))
    W2f = np.ascontiguousarray(np.asarray(inputs["W2"], np.float32))
    A4 = np.ascontiguousarray(np.stack(
        [np.asarray(inputs["a_src1"], np.float32),
         np.asarray(inputs["a_dst1"], np.float32),
         np.asarray(inputs["a_src2"], np.float32),
         np.asarray(inputs["a_dst2"], np.float32)], axis=1))
    base = {
        "XT": xT,
        "W1C": W1f.astype(ml_dtypes.bfloat16),
        "W2C": W2f.astype(ml_dtypes.bfloat16),
        "W1T": np.ascontiguousarray(W1f.T),
        "W2T": np.ascontiguousarray(W2f.T),
        "A4": A4,
        "B1": np.asarray(inputs["b1"], np.float32).reshape(1, OUT_C),
        "B2": np.asarray(inputs["b2"], np.float32).reshape(1, OUT_C),
        "BP1C": np.ascontiguousarray(
            np.asarray(inputs["bp1"], np.float32).reshape(8, P).T),
        "PRA": np.asarray(inputs["prelu_a"], np.float32).reshape(1, 1),
        "WP1": np.ascontiguousarray(
            np.asarray(inputs["Wp1"], np.float32)).astype(ml_dtypes.bfloat16),
        "WP2": np.ascontiguousarray(
            np.asarray(inputs["Wp2"], np.float32)).astype(ml_dtypes.bfloat16),
    }
    in_maps = []
    for c in range(NCORES):
        m = dict(base)
        m["MSK"] = np.array([[0.0 if c == NCORES - 1 else 1.0]], np.float32)
        m["EMTG"] = np.ascontiguousarray(emtg[c]).astype(ml_dtypes.bfloat16)
        m["EMTT"] = np.ascontiguousarray(emtt[c]).astype(ml_dtypes.bfloat16)
        m["SIDX16"] = np.ascontiguousarray(sidx16[c])
        m["DN16"] = np.ascontiguousarray(dn16[c])
        in_maps.append(m)

    if trace:
        trace = _ensure_trace_support()
    res = bass_utils.run_bass_kernel_spmd(
        nc, in_maps, core_ids=list(range(NCORES)), trace=trace,
        tmpdir=tmpdir)
    out = np.concatenate([res.results[c]["OUT"] for c in range(NCORES)],
                         axis=0)[:N]
    if debug:
        return out, res
    return out, res.exec_time_ns


def kernel(**inputs):
    out, _ = _run(inputs, trace=False)
    return out


# revision 19
# speedup vs baseline: 1.0372x; 1.0372x over previous
"""HANLayer (2x GATConv + semantic attention) Trainium2 Bass kernel, 8 cores.

Strategy (v3): edges sorted by dst, dst-node-sharded aggregation (1280 dst
nodes/core), replicated dense projection split per conv so conv2's projection
overlaps conv1's gather/aggregate.  Attention source logits are embedded in
the gathered H rows (width 1152 = 1024 h | 1 s | pad); destination logits are
expanded per chunk with a tiny matmul against a host-provided transposed
one-hot (EMTT), so each gather group needs exactly ONE dma_gather (1024 rows).
Per-128-edge one-hot matmuls do the softmax-weighted segment sum.
"""
import os
import sys

for _p in ("/opt/trn_rl_repo", "/root/.axon_site/_ro/trn_rl_repo"):
    if os.path.isdir(_p) and _p not in sys.path:
        sys.path.insert(0, _p)

import numpy as np
import ml_dtypes

import concourse.bacc as bacc
import concourse.bass as bass
import concourse.mybir as mybir
import concourse.tile as tile
from concourse import bass_utils, library_config
from concourse.masks import make_identity

F32 = mybir.dt.float32
BF16 = mybir.dt.bfloat16
I16 = mybir.dt.int16

N = 10000
E = 160000
IN_C = 512
OUT_C = 1024
NEG_SLOPE = 0.2
NCORES = 8
NPAD = 10240            # N padded to 80 blocks of 128
BPC = 10                # dst blocks per core
NODES_PER_CORE = 1280
P = 128
G = 8                   # chunks per gather group (1024 idx = HW limit)
NB = 80                 # node blocks
HW_W = 1152             # H row: 1024 h | 1 s | 127 pad (2304 B, /256 ok)

AddOp = mybir.AluOpType.add
SubOp = mybir.AluOpType.subtract
MulOp = mybir.AluOpType.mult
MaxOp = mybir.AluOpType.max


def _wrap_idx16(flat):
    """int16 index layout for dma_gather: idx i at [16*rep + i%16, i//16]."""
    n = flat.shape[0]
    assert n % 16 == 0
    w = flat.astype(np.int16).reshape(n // 16, 16).T  # [16, n//16]
    return np.tile(w, (8, 1))  # [128, n//16]


def _host_prep(edge_index):
    """Sort edges (plus self loops incl. pad nodes) by dst, chunk per core."""
    src = np.concatenate([edge_index[0].astype(np.int64),
                          np.arange(NPAD, dtype=np.int64)])
    dst = np.concatenate([edge_index[1].astype(np.int64),
                          np.arange(NPAD, dtype=np.int64)])
    order = np.argsort(dst, kind="stable")
    src_s = src[order]
    dst_s = dst[order]
    blk = dst_s // P
    counts = np.bincount(blk, minlength=NB)
    K = int(np.ceil(counts.max() / P))
    C = BPC * K
    C_pad = ((C + G - 1) // G) * G
    src_idx = np.zeros((NCORES, C_pad, P), np.int64)
    em = np.zeros((NCORES, C_pad, P, P), np.float32)
    bstart = np.searchsorted(blk, np.arange(NB + 1))
    for b in range(NB):
        core, bslot = divmod(b, BPC)
        lo, hi = bstart[b], bstart[b + 1]
        nb_ = hi - lo
        es = src_s[lo:hi]
        dloc = (dst_s[lo:hi] - b * P).astype(np.int64)
        for c in range((nb_ + P - 1) // P):
            e0 = c * P
            e1 = min(e0 + P, nb_)
            ci = bslot * K + c
            npts = e1 - e0
            src_idx[core, ci, :npts] = es[e0:e1]
            em[core, ci, np.arange(npts), dloc[e0:e1]] = 1.0
    NG = C_pad // G
    # group-major layouts:
    # EMTG[core, g, e, c*128+d] = em[core, g*G+c, e, d]
    emtg = np.ascontiguousarray(
        em.reshape(NCORES, NG, G, P, P).transpose(0, 1, 3, 2, 4)
        .reshape(NCORES, NG, P, G * P))
    # EMTT[core, g, d, c*128+e] = em[core, g*G+c, e, d]
    emtt = np.ascontiguousarray(
        em.transpose(0, 1, 3, 2).reshape(NCORES, NG, G, P, P)
        .transpose(0, 1, 3, 2, 4).reshape(NCORES, NG, P, G * P))
    sidx16 = np.stack([_wrap_idx16(src_idx[c].reshape(-1))
                       for c in range(NCORES)])
    dn16 = np.stack([_wrap_idx16(np.arange(c * NODES_PER_CORE,
                                           (c + 1) * NODES_PER_CORE))
                     for c in range(NCORES)])
    return K, sidx16, emtg, emtt, dn16


def _build_program(K, debug=False):
    C = BPC * K
    C_pad = ((C + G - 1) // G) * G
    NG = C_pad // G
    KIND_DBG = "ExternalOutput" if debug else "Internal"
    nc = bacc.Bacc("TRN2", target_bir_lowering=False, debug=False,
                   enable_asserts=False, num_devices=NCORES)

    # ---- inputs (replicated except EMTG/EMTT/SIDX16/MSK) ----
    XT = nc.dram_tensor("XT", [IN_C, NPAD], BF16, kind="ExternalInput")
    W1C = nc.dram_tensor("W1C", [IN_C, OUT_C], BF16, kind="ExternalInput")
    W2C = nc.dram_tensor("W2C", [IN_C, OUT_C], BF16, kind="ExternalInput")
    W1T = nc.dram_tensor("W1T", [OUT_C, IN_C], F32, kind="ExternalInput")
    W2T = nc.dram_tensor("W2T", [OUT_C, IN_C], F32, kind="ExternalInput")
    A4 = nc.dram_tensor("A4", [OUT_C, 4], F32, kind="ExternalInput")
    B1 = nc.dram_tensor("B1", [1, OUT_C], F32, kind="ExternalInput")
    B2 = nc.dram_tensor("B2", [1, OUT_C], F32, kind="ExternalInput")
    BP1C = nc.dram_tensor("BP1C", [P, 8], F32, kind="ExternalInput")
    PRA = nc.dram_tensor("PRA", [1, 1], F32, kind="ExternalInput")
    MSK = nc.dram_tensor("MSK", [1, 1], F32, kind="ExternalInput")
    WP1 = nc.dram_tensor("WP1", [OUT_C, OUT_C], BF16, kind="ExternalInput")
    WP2 = nc.dram_tensor("WP2", [OUT_C, OUT_C], BF16, kind="ExternalInput")
    EMTG = nc.dram_tensor("EMTG", [NG, P, G * P], BF16, kind="ExternalInput")
    EMTT = nc.dram_tensor("EMTT", [NG, P, G * P], BF16, kind="ExternalInput")
    SIDX16 = nc.dram_tensor("SIDX16", [P, C_pad * 8], I16,
                            kind="ExternalInput")
    DN16 = nc.dram_tensor("DN16", [P, BPC * 8], I16, kind="ExternalInput")

    OUT = nc.dram_tensor("OUT", [NODES_PER_CORE, OUT_C], F32,
                         kind="ExternalOutput")

    # ---- internal DRAM ----
    H1 = nc.dram_tensor("H1", [NPAD, HW_W], BF16, kind=KIND_DBG)
    H2 = nc.dram_tensor("H2", [NPAD, HW_W], BF16, kind=KIND_DBG)
    S4 = nc.dram_tensor("S4", [NPAD, 64], F32, kind=KIND_DBG)
    ARIN = nc.dram_tensor("ARIN", [OUT_C], F32, kind="Internal")
    AROUT = nc.dram_tensor("AROUT", [OUT_C], F32, kind="Internal",
                           addr_space="Shared")
    ATTD = nc.dram_tensor("ATTD", [1, OUT_C], F32, kind=KIND_DBG)
    if debug:
        DBGH1 = nc.dram_tensor("DBGH1", [NODES_PER_CORE, OUT_C], F32,
                               kind="ExternalOutput")
        DBGH2 = nc.dram_tensor("DBGH2", [NODES_PER_CORE, OUT_C], F32,
                               kind="ExternalOutput")

    NKC = IN_C // P  # 4 k-chunks of input features

    with tile.TileContext(nc) as tc:
        with tc.tile_pool(name="persist", bufs=1) as pp:
            nc.gpsimd.load_library(library_config.mlp)
            b1b = pp.tile([P, OUT_C], F32, tag="b1b")
            b2b = pp.tile([P, OUT_C], F32, tag="b2b")
            nc.sync.dma_start(b1b[:], B1.ap().to_broadcast((P, OUT_C)))
            nc.sync.dma_start(b2b[:], B2.ap().to_broadcast((P, OUT_C)))
            bp1c = pp.tile([P, 8], F32, tag="bp1c")
            nc.sync.dma_start(bp1c[:], BP1C.ap())
            pa_col = pp.tile([P, 1], F32, tag="pa_col")
            nc.sync.dma_start(pa_col[:], PRA.ap().to_broadcast((P, 1)))
            msk_col = pp.tile([P, 1], F32, tag="msk_col")
            nc.sync.dma_start(msk_col[:], MSK.ap().to_broadcast((P, 1)))
            ones = pp.tile([P, 1], BF16, tag="ones")
            nc.vector.memset(ones[:], 1.0)
            identity = pp.tile([P, P], F32, tag="identity")
            make_identity(nc, identity[:])
            identity16 = pp.tile([P, P], BF16, tag="identity16")
            nc.vector.tensor_copy(identity16[:], identity[:])
            sidx16 = pp.tile([P, C_pad * 8], I16, tag="sidx16")
            nc.sync.dma_start(sidx16[:], SIDX16.ap())
            dn16 = pp.tile([P, BPC * 8], I16, tag="dn16")
            nc.sync.dma_start(dn16[:], DN16.ap())
            acc1 = pp.tile([P, BPC * OUT_C], F32, tag="acc1")
            acc2 = pp.tile([P, BPC * OUT_C], F32, tag="acc2")
            tbcols = pp.tile([P, 96], F32, tag="tbcols")
            wt_g = [pp.tile([P, 4], BF16, tag=f"wt{g}", name=f"wt{g}")
                    for g in range(NKC)]

            # ================= wtilde = [W1a, W2a] =================
            with tc.tile_pool(name="wt", bufs=1) as sp, \
                 tc.tile_pool(name="wtps", bufs=1, space="PSUM") as psp:
                a4t = [sp.tile([P, 4], F32, tag="a4", bufs=8, name=f"a4_{oc}")
                       for oc in range(8)]
                for oc in range(8):
                    nc.sync.dma_start(a4t[oc][:],
                                      A4.ap()[oc * P:(oc + 1) * P, :])
                for g in range(NKC):
                    pwt1 = psp.tile([P, 2], F32, tag="pwt", bufs=4)
                    pwt2 = psp.tile([P, 2], F32, tag="pwt", bufs=4)
                    for oc in range(8):
                        w1t_t = sp.tile([P, P], F32, tag="w1t", bufs=3)
                        nc.sync.dma_start(
                            w1t_t[:], W1T.ap()[oc * P:(oc + 1) * P,
                                               g * P:(g + 1) * P])
                        nc.tensor.matmul(pwt1[:], lhsT=w1t_t[:],
                                         rhs=a4t[oc][:, 0:2],
                                         start=(oc == 0), stop=(oc == 7))
                        w2t_t = sp.tile([P, P], F32, tag="w2t", bufs=3)
                        nc.sync.dma_start(
                            w2t_t[:], W2T.ap()[oc * P:(oc + 1) * P,
                                               g * P:(g + 1) * P])
                        nc.tensor.matmul(pwt2[:], lhsT=w2t_t[:],
                                         rhs=a4t[oc][:, 2:4],
                                         start=(oc == 0), stop=(oc == 7))
                    nc.vector.tensor_copy(wt_g[g][:, 0:2], pwt1[:])
                    nc.vector.tensor_copy(wt_g[g][:, 2:4], pwt2[:])

            # ========== P1a: conv1 projection + S ==========
            with tc.tile_pool(name="p1a", bufs=1) as sp, \
                 tc.tile_pool(name="p1aps", bufs=1, space="PSUM") as psp:
                rhs1 = [sp.tile([P, OUT_C], BF16, tag=f"rhs1_{g}",
                                name=f"rhs1_{g}") for g in range(NKC)]
                for g in range(NKC):
                    nc.sync.dma_start(rhs1[g][:],
                                      W1C.ap()[g * P:(g + 1) * P, :])
                xtc = None
                for i in range(NB):
                    i4, j = divmod(i, 4)
                    if j == 0:
                        xtc = sp.tile([P, NKC, 4 * P], BF16, tag="xtc",
                                      bufs=2)
                        nc.sync.dma_start(
                            xtc[:, :, :],
                            bass.AP(XT, i4 * 4 * P,
                                    [[NPAD, P], [P * NPAD, NKC], [1, 4 * P]]))
                    ps_h = psp.tile([P, OUT_C], F32, tag="ph", bufs=2)
                    ps_s = psp.tile([P, 4], F32, tag="ps", bufs=2)
                    for g in range(NKC):
                        lh = xtc[:, g, j * P:(j + 1) * P]
                        nc.tensor.matmul(ps_h[:, 0:512], lhsT=lh,
                                         rhs=rhs1[g][:, 0:512],
                                         start=(g == 0), stop=(g == NKC - 1))
                        nc.tensor.matmul(ps_h[:, 512:1024], lhsT=lh,
                                         rhs=rhs1[g][:, 512:1024],
                                         start=(g == 0), stop=(g == NKC - 1))
                        nc.tensor.matmul(ps_s[:], lhsT=lh,
                                         rhs=wt_g[g][:, 0:4],
                                         start=(g == 0), stop=(g == NKC - 1))
                    stg = sp.tile([P, HW_W], BF16, tag="stg", bufs=3)
                    nc.vector.memset(stg[:, OUT_C + 1:HW_W], 0.0)
                    nc.scalar.activation(stg[:, 0:OUT_C], ps_h[:],
                                         mybir.ActivationFunctionType.Copy)
                    nc.vector.tensor_copy(stg[:, OUT_C:OUT_C + 1],
                                          ps_s[:, 0:1])
                    s64 = sp.tile([P, 64], F32, tag="s64", bufs=3)
                    nc.vector.memset(s64[:, 4:64], 0.0)
                    nc.vector.tensor_copy(s64[:, 0:4], ps_s[:])
                    rows = slice(i * P, (i + 1) * P)
                    nc.sync.dma_start(H1.ap()[rows, :], stg[:])
                    nc.sync.dma_start(S4.ap()[rows, :], s64[:])

            # ========== agg phase (shared for both convs) ==========
            def agg_phase(sp, psp, Hsrc, soff, accT, bbT, proj_emitter,
                          proj_total, post_block):
                proj_done = [0]
                num = [None]
                den = [None]
                sbd = sp.tile([P, BPC, 64], F32, tag="sbd", name="sbd")
                nc.gpsimd.dma_gather(
                    sbd[:, 0:5, :], S4.ap(), dn16[:, 0:40],
                    5 * P, 5 * P, 64)
                nc.gpsimd.dma_gather(
                    sbd[:, 5:10, :], S4.ap(), dn16[:, 40:80],
                    5 * P, 5 * P, 64)
                sb16 = sp.tile([P, BPC], BF16, tag="sb16", name="sb16")
                nc.vector.tensor_copy(sb16[:], sbd[:, :, soff:soff + 1])
                tiles = {}

                def stage_a(gi):
                    hg = sp.tile([P, G, HW_W], BF16, tag="hg", bufs=2,
                                 name="hg")
                    nc.gpsimd.dma_gather(
                        hg[:, :, :], Hsrc.ap(),
                        sidx16[:, gi * G * 8:(gi + 1) * G * 8],
                        G * P, G * P, HW_W)
                    emg = sp.tile([P, G, P], BF16, tag="emg", bufs=2,
                                  name="emg")
                    nc.scalar.dma_start(emg[:, :, :], EMTG.ap()[gi])
                    emt = sp.tile([P, G * P], BF16, tag="emt", bufs=2,
                                  name="emt")
                    nc.scalar.dma_start(emt[:], EMTT.ap()[gi])
                    dvps = psp.tile([P, G], F32, tag="dvps", bufs=1,
                                    name="dvps")
                    nc.vector.memset(dvps[:], 0.0)
                    for c in range(G):
                        ci = gi * G + c
                        if ci >= C:
                            continue
                        b_ = ci // K
                        nc.tensor.matmul(dvps[:, c:c + 1],
                                         lhsT=emt[:, c * P:(c + 1) * P],
                                         rhs=sb16[:, b_:b_ + 1],
                                         start=True, stop=True)
                    e2 = sp.tile([P, G], F32, tag="e2", bufs=2, name="e2")
                    nc.vector.tensor_tensor(
                        out=e2[:], in0=hg[:, :, OUT_C:OUT_C + 1],
                        in1=dvps[:], op=AddOp)
                    lr = sp.tile([P, G], F32, tag="lr", bufs=2, name="lr")
                    nc.vector.scalar_tensor_tensor(
                        out=lr[:], in0=e2[:], scalar=NEG_SLOPE, in1=e2[:],
                        op0=MulOp, op1=MaxOp)
                    al = sp.tile([P, G], F32, tag="al", bufs=2, name="al")
                    nc.scalar.activation(al[:], lr[:],
                                         mybir.ActivationFunctionType.Exp)
                    tiles[gi] = (hg, emg, al)

                stage_a(0)
                for gi in range(NG):
                    target = min(proj_total,
                                 int(np.ceil(proj_total * (gi + 1) / NG)))
                    while proj_done[0] < target:
                        proj_emitter(proj_done[0])
                        proj_done[0] += 1
                    if gi + 1 < NG:
                        stage_a(gi + 1)
                    hg, emg, al = tiles.pop(gi)
                    amg = sp.tile([P, G, P], BF16, tag="amg", bufs=2,
                                  name="amg")
                    nc.vector.tensor_tensor(
                        out=amg[:, :, :], in0=emg[:, :, :],
                        in1=al[:].to_broadcast((P, G, P)), op=MulOp)
                    for c in range(G):
                        ci = gi * G + c
                        if ci >= C:
                            continue
                        b, k = divmod(ci, K)
                        if k == 0:
                            num[0] = psp.tile([P, OUT_C], F32, tag="num",
                                              bufs=1, name="num")
                            den[0] = psp.tile([P, 1], F32, tag="den", bufs=1,
                                              name="den")
                        nc.tensor.matmul(num[0][:, 0:512],
                                         lhsT=amg[:, c, :],
                                         rhs=hg[:, c, 0:512],
                                         start=(k == 0), stop=(k == K - 1))
                        nc.tensor.matmul(num[0][:, 512:1024],
                                         lhsT=amg[:, c, :],
                                         rhs=hg[:, c, 512:1024],
                                         start=(k == 0), stop=(k == K - 1))
                        nc.tensor.matmul(den[0][:], lhsT=amg[:, c, :],
                                         rhs=ones[:],
                                         start=(k == 0), stop=(k == K - 1))
                        if k == K - 1:
                            rden = sp.tile([P, 1], F32, tag="rden", bufs=2)
                            nc.vector.reciprocal(rden[:], den[0][:])
                            tmp = sp.tile([P, OUT_C], F32, tag="gtmp", bufs=2)
                            nc.vector.scalar_tensor_tensor(
                                out=tmp[:], in0=num[0][:],
                                scalar=rden[:, 0:1], in1=bbT[:],
                                op0=MulOp, op1=AddOp)
                            hcols = slice(b * OUT_C, (b + 1) * OUT_C)
                            nc.vector.scalar_tensor_tensor(
                                out=accT[:, hcols], in0=tmp[:],
                                scalar=pa_col[:, 0:1], in1=tmp[:],
                                op0=MulOp, op1=MaxOp)
                            post_block(b)
                while proj_done[0] < proj_total:
                    proj_emitter(proj_done[0])
                    proj_done[0] += 1

            # ========== P2a (conv1 agg) overlapped with P1b (conv2 proj) ====
            with tc.tile_pool(name="p2a", bufs=1) as sp, \
                 tc.tile_pool(name="p2aps", bufs=1, space="PSUM") as psp:
                rhs2 = [sp.tile([P, OUT_C], BF16, tag=f"rhs2_{g}",
                                name=f"rhs2_{g}") for g in range(NKC)]
                for g in range(NKC):
                    nc.sync.dma_start(rhs2[g][:],
                                      W2C.ap()[g * P:(g + 1) * P, :])
                xtc2 = [None]

                def proj2(i):
                    i4, j = divmod(i, 4)
                    if j == 0:
                        xtc2[0] = sp.tile([P, NKC, 4 * P], BF16, tag="xtc2",
                                          bufs=2, name="xtc2")
                        nc.sync.dma_start(
                            xtc2[0][:, :, :],
                            bass.AP(XT, i4 * 4 * P,
                                    [[NPAD, P], [P * NPAD, NKC], [1, 4 * P]]))
                    ps_h = psp.tile([P, OUT_C], F32, tag="ph2", bufs=2)
                    for g in range(NKC):
                        lh = xtc2[0][:, g, j * P:(j + 1) * P]
                        nc.tensor.matmul(ps_h[:, 0:512], lhsT=lh,
                                         rhs=rhs2[g][:, 0:512],
                                         start=(g == 0), stop=(g == NKC - 1))
                        nc.tensor.matmul(ps_h[:, 512:1024], lhsT=lh,
                                         rhs=rhs2[g][:, 512:1024],
                                         start=(g == 0), stop=(g == NKC - 1))
                    stg = sp.tile([P, HW_W], BF16, tag="stg2", bufs=3,
                                  name="stg2")
                    s2col = sp.tile([P, 1], F32, tag="s2col", bufs=2,
                                    name="s2col")
                    nc.sync.dma_start(s2col[:],
                                      S4.ap()[i * P:(i + 1) * P, 2:3])
                    nc.vector.memset(stg[:, OUT_C + 1:HW_W], 0.0)
                    nc.scalar.activation(stg[:, 0:OUT_C], ps_h[:],
                                         mybir.ActivationFunctionType.Copy)
                    nc.vector.tensor_copy(stg[:, OUT_C:OUT_C + 1], s2col[:])
                    nc.sync.dma_start(H2.ap()[i * P:(i + 1) * P, :], stg[:])

                agg_phase(sp, psp, H1, 1, acc1, b1b, proj2, NB,
                          lambda b: None)

            # ========== P2b (conv2 agg) overlapped with P3 per-block ========
            with tc.tile_pool(name="p2b", bufs=1) as sp, \
                 tc.tile_pool(name="p2bps", bufs=1, space="PSUM") as psp:
                wp1k = [sp.tile([P, OUT_C], BF16, tag=f"wp1_{k}",
                                name=f"wp1_{k}") for k in range(8)]
                for k in range(8):
                    nc.scalar.dma_start(wp1k[k][:],
                                        WP1.ap()[k * P:(k + 1) * P, :])

                def p3_block(b):
                    hcols = slice(b * OUT_C, (b + 1) * OUT_C)
                    hsb = sp.tile([P, OUT_C], BF16, tag="hsb", bufs=2)
                    nc.vector.tensor_tensor(out=hsb[:], in0=acc1[:, hcols],
                                            in1=acc2[:, hcols], op=AddOp)
                    htb = []
                    for kk in range(8):
                        tp = psp.tile([P, P], BF16, tag="tp", bufs=2)
                        nc.tensor.transpose(tp[:],
                                            hsb[:, kk * P:(kk + 1) * P],
                                            identity16[:])
                        ht = sp.tile([P, P], BF16, tag=f"htb{kk}", bufs=2,
                                     name=f"htb{kk}")
                        nc.vector.tensor_copy(ht[:], tp[:])
                        htb.append(ht)
                    for o2 in range(8):
                        wps = psp.tile([P, P], F32, tag="wps", bufs=2)
                        for kk in range(8):
                            nc.tensor.matmul(
                                wps[:],
                                lhsT=wp1k[kk][:, o2 * P:(o2 + 1) * P],
                                rhs=htb[kk][:],
                                start=(kk == 0), stop=(kk == 7))
                        scr = sp.tile([P, P], BF16, tag="scr", bufs=2)
                        col0 = o2 * 12
                        if b < 8:
                            nc.scalar.activation(
                                scr[:], wps[:],
                                mybir.ActivationFunctionType.Tanh,
                                bias=bp1c[:, o2:o2 + 1],
                                accum_out=tbcols[:, col0 + b:col0 + b + 1])
                        elif b == 8:
                            nc.scalar.activation(
                                scr[:, 0:16], wps[:, 0:16],
                                mybir.ActivationFunctionType.Tanh,
                                bias=bp1c[:, o2:o2 + 1],
                                accum_out=tbcols[:, col0 + 8:col0 + 9])
                            nc.scalar.activation(
                                scr[:, 16:P], wps[:, 16:P],
                                mybir.ActivationFunctionType.Tanh,
                                bias=bp1c[:, o2:o2 + 1],
                                accum_out=tbcols[:, col0 + 9:col0 + 10])
                        else:
                            nc.scalar.activation(
                                scr[:], wps[:],
                                mybir.ActivationFunctionType.Tanh,
                                bias=bp1c[:, o2:o2 + 1],
                                accum_out=tbcols[:, col0 + 10:col0 + 11])

                agg_phase(sp, psp, H2, 3, acc2, b2b, lambda i: None, 0,
                          p3_block)

            # ========== tail: semantic attention + blend ==========
            with tc.tile_pool(name="tail", bufs=1) as sp, \
                 tc.tile_pool(name="tailps", bufs=1, space="PSUM") as psp:
                wp2k = [sp.tile([P, OUT_C], BF16, tag=f"wp2_{k}",
                                name=f"wp2_{k}") for k in range(8)]
                for k in range(8):
                    nc.scalar.dma_start(wp2k[k][:],
                                        WP2.ap()[k * P:(k + 1) * P, :])
                tbar = sp.tile([P, 8], F32, tag="tbar")
                for o2 in range(8):
                    col0 = o2 * 12
                    tmain = sp.tile([P, 1], F32, tag="tmain", bufs=2)
                    nc.vector.tensor_reduce(
                        tmain[:], tbcols[:, col0:col0 + 9],
                        mybir.AxisListType.X, AddOp)
                    tmsk = sp.tile([P, 1], F32, tag="tmsk", bufs=2)
                    nc.vector.tensor_reduce(
                        tmsk[:], tbcols[:, col0 + 9:col0 + 11],
                        mybir.AxisListType.X, AddOp)
                    nc.vector.scalar_tensor_tensor(
                        out=tbar[:, o2:o2 + 1], in0=tmsk[:],
                        scalar=msk_col[:, 0:1], in1=tmain[:],
                        op0=MulOp, op1=AddOp)
                arview = [[1, P], [P, 8]]
                nc.sync.dma_start(bass.AP(ARIN, 0, arview), tbar[:])
                nc.gpsimd.load_library(library_config.standard)
                nc.gpsimd.collective_compute(
                    "AllReduce", AddOp,
                    replica_groups=[list(range(NCORES))],
                    ins=[ARIN.ap().opt()], outs=[AROUT.ap().opt()])
                tbm = sp.tile([P, 8], F32, tag="tbm")
                nc.sync.dma_start(tbm[:], bass.AP(AROUT, 0, arview))
                tbn = sp.tile([P, 8], BF16, tag="tbn")
                nc.vector.tensor_scalar_mul(tbn[:], tbm[:], 1.0 / N)
                pw = psp.tile([1, OUT_C], F32, tag="pw")
                for kk in range(8):
                    nc.tensor.matmul(pw[:, 0:512], lhsT=tbn[:, kk:kk + 1],
                                     rhs=wp2k[kk][:, 0:512], start=(kk == 0),
                                     stop=(kk == 7))
                    nc.tensor.matmul(pw[:, 512:1024], lhsT=tbn[:, kk:kk + 1],
                                     rhs=wp2k[kk][:, 512:1024],
                                     start=(kk == 0), stop=(kk == 7))
                et = sp.tile([1, OUT_C], F32, tag="et")
                esum = sp.tile([1, 1], F32, tag="esum")
                nc.scalar.activation(et[:], pw[:],
                                     mybir.ActivationFunctionType.Exp,
                                     accum_out=esum[:])
                rs = sp.tile([1, 1], F32, tag="rs")
                nc.vector.reciprocal(rs[:], esum[:])
                att1 = sp.tile([1, OUT_C], F32, tag="att1")
                nc.vector.tensor_scalar_mul(att1[:], et[:], rs[:, 0:1])
                nc.sync.dma_start(ATTD.ap(), att1[:])
                attb = sp.tile([P, OUT_C], F32, tag="attb")
                nc.sync.dma_start(attb[:],
                                  ATTD.ap().to_broadcast((P, OUT_C)))
                for b in range(BPC):
                    hcols = slice(b * OUT_C, (b + 1) * OUT_C)
                    d = sp.tile([P, OUT_C], F32, tag="bd", bufs=2)
                    nc.vector.tensor_tensor(out=d[:], in0=acc1[:, hcols],
                                            in1=acc2[:, hcols], op=SubOp)
                    m = sp.tile([P, OUT_C], F32, tag="bm", bufs=2)
                    nc.vector.tensor_tensor(out=m[:], in0=d[:], in1=attb[:],
                                            op=MulOp)
                    o = sp.tile([P, OUT_C], F32, tag="bo", bufs=2)
                    nc.vector.tensor_tensor(out=o[:], in0=m[:],
                                            in1=acc2[:, hcols], op=AddOp)
                    nc.sync.dma_start(OUT.ap()[b * P:(b + 1) * P, :], o[:])
                if debug:
                    for b in range(BPC):
                        hcols = slice(b * OUT_C, (b + 1) * OUT_C)
                        nc.sync.dma_start(
                            DBGH1.ap()[b * P:(b + 1) * P, :], acc1[:, hcols])
                        nc.sync.dma_start(
                            DBGH2.ap()[b * P:(b + 1) * P, :], acc2[:, hcols])

    nc.compile()
    return nc


_PROG_CACHE = {}


def _ensure_trace_support():
    """Install the missing antenv.axon_hooks NTFF shim so trace=True works."""
    import types
    try:
        from antenv import axon_hooks  # noqa: F401
        return True
    except ImportError:
        pass
    try:
        import antenv
        if "/root/.axon_site" not in sys.path:
            sys.path.append("/root/.axon_site")
        from trn_agent_boot.trn_boot import _ntff_profile_via_ctypes
        hook = _ntff_profile_via_ctypes("/opt/axon/libaxon_pjrt.so")
        if hook is None:
            return False
        mod = types.ModuleType("antenv.axon_hooks")
        mod._hook = hook
        mod.get_axon_ntff_profile_hook = lambda: mod._hook
        mod.set_axon_ntff_profile_hook = lambda h: setattr(mod, "_hook", h)
        sys.modules["antenv.axon_hooks"] = mod
        antenv.axon_hooks = mod
        bass_utils.upload_artifacts = lambda t: str(t)
        return True
    except Exception as e:  # noqa: BLE001
        print("trace support unavailable:", e)
        return False


def _get_program(K):
    if K not in _PROG_CACHE:
        _PROG_CACHE[K] = _build_program(K)
    return _PROG_CACHE[K]


def _run(inputs, trace=False, debug=False, tmpdir=None):
    x = np.asarray(inputs["x"], np.float32)
    edge_index = np.asarray(inputs["edge_index"])
    K, sidx16, emtg, emtt, dn16 = _host_prep(edge_index)
    if debug:
        nc = _build_program(K, debug=True)
    else:
        nc = _get_program(K)

    xpad = np.zeros((NPAD, IN_C), np.float32)
    xpad[:N] = x
    xT = np.ascontiguousarray(xpad.T).astype(ml_dtypes.bfloat16)
    W1f = np.ascontiguousarray(np.asarray(inputs["W1"], np.float32))
    W2f = np.ascontiguousarray(np.asarray(inputs["W2"], np.float32))
    A4 = np.ascontiguousarray(np.stack(
        [np.asarray(inputs["a_src1"], np.float32),
         np.asarray(inputs["a_dst1"], np.float32),
         np.asarray(inputs["a_src2"], np.float32),
         np.asarray(inputs["a_dst2"], np.float32)], axis=1))
    base = {
        "XT": xT,
        "W1C": W1f.astype(ml_dtypes.bfloat16),
        "W2C": W2f.astype(ml_dtypes.bfloat16),
        "W1T": np.ascontiguousarray(W1f.T),
        "W2T": np.ascontiguousarray(W2f.T),
        "A4": A4,
        "B1": np.asarray(inputs["b1"], np.float32).reshape(1, OUT_C),
        "B2": np.asarray(inputs["b2"], np.float32).reshape(1, OUT_C),
        "BP1C": np.ascontiguousarray(
            np.asarray(inputs["bp1"], np.float32).reshape(8, P).T),
        "PRA": np.asarray(inputs["prelu_a"], np.float32).reshape(1, 1),
        "WP1": np.ascontiguousarray(
            np.asarray(inputs["Wp1"], np.float32)).astype(ml_dtypes.bfloat16),
        "WP2": np.ascontiguousarray(
            np.asarray(inputs["Wp2"], np.float32)).astype(ml_dtypes.bfloat16),
    }
    in_maps = []
    for c in range(NCORES):
        m = dict(base)
        m["MSK"] = np.array([[0.0 if c == NCORES - 1 else 1.0]], np.float32)
        m["EMTG"] = np.ascontiguousarray(emtg[c]).astype(ml_dtypes.bfloat16)
        m["EMTT"] = np.ascontiguousarray(emtt[c]).astype(ml_dtypes.bfloat16)
        m["SIDX16"] = np.ascontiguousarray(sidx16[c])
        m["DN16"] = np.ascontiguousarray(dn16[c])
        in_maps.append(m)

    if trace:
        trace = _ensure_trace_support()
    res = bass_utils.run_bass_kernel_spmd(
        nc, in_maps, core_ids=list(range(NCORES)), trace=trace,
        tmpdir=tmpdir)
    out = np.concatenate([res.results[c]["OUT"] for c in range(NCORES)],
                         axis=0)[:N]
    if debug:
        return out, res
    return out, res.exec_time_ns


def kernel(**inputs):
    out, _ = _run(inputs, trace=False)
    return out
